# revision 17
# baseline (speedup 1.0000x reference)
"""ChannelMoE Trainium2 kernel (fp16 streaming version).

Computes, per batch element b:
    pool   = mean(x[b], axis=-1)                               [C]
    h      = relu(pool[:,None]*w1 + b1)                        [C,4]
    scores = einsum('ij,ioj->io', h, w2) + b2                  [C,C]
    s      = layernorm(scores)*gamma + beta, then / temperature
    mask   = top-4 of each row (ties resolved to lowest index, as
             jax.lax.top_k does) via max8 + match_replace
    W      = softmax of masked s per row (zeros elsewhere)
    out[b] = (W + I) @ x[b]          # identity folds in the +x residual

Sharding: data-parallel over B across 8 NeuronCores (8 batch elements
per core); the small weight-gen / norm params are replicated.

Precision: x is converted to fp16 on the host (halving DMA traffic both
directions and running the channel-mix matmuls at 1 cycle/row instead of
fp32's 4); all weight-generation math stays fp32 on-device.  fp16 (not
bf16) keeps the pooled means accurate enough that the top-4 selection
matches the fp32 reference exactly (measured 0 flipped rows, rel err
3.3e-4 vs the 2e-2 budget; bf16 flips ~5 rows and lands at 1.5e-2).

Structure: one software-pipelined pass per batch element: DMA-in
(alternating SP HWDGE / SWDGE rings), pooling split across DVE+ACT+GPSIMD,
the fp32 weight-gen chain spread over DVE and GPSIMD with exp/sqrt on ACT,
PE transpose, 8 fp16 matmuls, PSUM evacuation split ACT/DVE, DMA-out
(alternating ACT/DVE HWDGE rings).  Batches overlap through the Tile
dependency tracker, so DMA streams continuously while compute runs.
"""

import numpy as np

import concourse.bacc as bacc
import concourse.bass as bass
import concourse.tile as tile
from concourse import masks, mybir
from concourse.bass_utils import run_bass_kernel_spmd

B, C, L, K = 64, 128, 4096, 4
NCORES = 8
BS = B // NCORES
EPS = 1e-5
F32 = mybir.dt.float32
F16 = mybir.dt.float16
MM_CHUNK = 512
NCHUNK = L // MM_CHUNK  # 8 matmul chunks per batch row

_NC = None


def _emit(nc, reps=1):
    x = nc.dram_tensor("x", [BS, C, L], F16, kind="ExternalInput").ap()
    w1 = nc.dram_tensor("w1", [C, 4], F32, kind="ExternalInput").ap()
    b1 = nc.dram_tensor("b1", [C, 4], F32, kind="ExternalInput").ap()
    w2 = nc.dram_tensor("w2", [C, C, 4], F32, kind="ExternalInput").ap()
    b2 = nc.dram_tensor("b2", [C, C], F32, kind="ExternalInput").ap()
    gamma = nc.dram_tensor("gamma", [C], F32, kind="ExternalInput").ap()
    beta = nc.dram_tensor("beta", [C], F32, kind="ExternalInput").ap()
    temp = nc.dram_tensor("temperature", [1], F32, kind="ExternalInput").ap()
    out = nc.dram_tensor("out", [BS, C, L], F16, kind="ExternalOutput").ap()

    def bcast_over_partitions(ap, n=C):
        # [F] dram vector -> [n, F] with partition stride 0
        return bass.AP(tensor=ap.tensor, offset=ap.offset, ap=[[0, n]] + list(ap.ap))

    with tile.TileContext(nc) as tc:
        with (
            tc.tile_pool(name="const", bufs=1) as const,
            tc.tile_pool(name="xin", bufs=BS) as xin,
            tc.tile_pool(name="oout", bufs=3) as oout,
            tc.tile_pool(name="wg", bufs=2) as wg,
            tc.tile_pool(name="wts", bufs=BS) as wts,
            tc.tile_pool(name="sm", bufs=4) as sm,
            tc.tile_pool(name="psmm", bufs=3, space="PSUM") as psmm,
            tc.tile_pool(name="pstr", bufs=2, space="PSUM") as pstr,
        ):
            # ---- one-time constants ----
            w1_sb = const.tile([C, 4], F32)
            nc.sync.dma_start(out=w1_sb, in_=w1)
            b1_sb = const.tile([C, 4], F32)
            nc.sync.dma_start(out=b1_sb, in_=b1)
            w2_sb = const.tile([C, C, 4], F32)
            nc.sync.dma_start(out=w2_sb, in_=w2)
            b2_sb = const.tile([C, C], F32)
            nc.sync.dma_start(out=b2_sb, in_=b2)
            gamma_sb = const.tile([C, C], F32)
            nc.gpsimd.dma_start(out=gamma_sb, in_=bcast_over_partitions(gamma))
            beta_sb = const.tile([C, C], F32)
            nc.gpsimd.dma_start(out=beta_sb, in_=bcast_over_partitions(beta))
            temp_sb = const.tile([C, 1], F32)
            nc.gpsimd.dma_start(out=temp_sb, in_=bcast_over_partitions(temp))

            identity = const.tile([C, C], F32)
            masks.make_identity(nc, identity[:])

            eps_sb = const.tile([C, 1], F32)
            nc.vector.memset(eps_sb, EPS)

            # per-batch top-4 match slots, upper half stays -1e38 forever
            t4p = const.tile([C, 8 * BS], F32)
            nc.vector.memset(t4p, -1e38)

            rtemp = const.tile([C, 1], F32)
            nc.vector.reciprocal(rtemp, temp_sb)
            # fold 1/temperature into gamma/beta, 1/L into w1
            nc.vector.tensor_scalar_mul(gamma_sb[:], gamma_sb[:], rtemp[:, 0:1])
            nc.vector.tensor_scalar_mul(beta_sb[:], beta_sb[:], rtemp[:, 0:1])
            nc.vector.tensor_scalar_mul(w1_sb[:], w1_sb[:], 1.0 / L)

            # `reps` > 1 repeats the whole computation inside one NEFF —
            # used only by the timing harness to isolate exec time from
            # per-dispatch overhead.  The graded kernel uses reps=1.
            for _rep in range(reps):
              for b in range(BS):
                  # ---- stream in x[b] (fp16, 1 MiB) ----
                  x_t = xin.tile([C, L], F16, tag="x")
                  (nc.sync if b % 2 == 0 else nc.gpsimd).dma_start(
                      out=x_t, in_=x[b])

                  # ---- pooling: one fp16 2x-mode reduce on DVE ----
                  pool_s = sm.tile([C, 1], F32)
                  nc.vector.tensor_reduce(
                      out=pool_s, in_=x_t, axis=mybir.AxisListType.X,
                      op=mybir.AluOpType.add,
                  )

                  # ---- h = relu(pool*w1 + b1)  (w1 pre-scaled by 1/L) ----
                  h = sm.tile([C, 4], F32)
                  nc.vector.scalar_tensor_tensor(
                      out=h, in0=w1_sb[:], scalar=pool_s[:, 0:1], in1=b1_sb[:],
                      op0=mybir.AluOpType.mult, op1=mybir.AluOpType.add,
                  )
                  nc.gpsimd.tensor_scalar_max(h, h, 0.0)

                  # ---- scores = einsum('ij,ioj->io', h, w2) + b2 ----
                  scores = wg.tile([C, C], F32)
                  nc.vector.scalar_tensor_tensor(
                      out=scores, in0=w2_sb[:, :, 0], scalar=h[:, 0:1],
                      in1=b2_sb[:], op0=mybir.AluOpType.mult,
                      op1=mybir.AluOpType.add,
                  )
                  # per-partition (AP) scalars are DVE-only: gpsimd lacks
                  # the TensorScalarPtr / ScalarTensorTensorPtr opcodes
                  for j in (1, 2, 3):
                      nc.vector.scalar_tensor_tensor(
                          out=scores, in0=w2_sb[:, :, j], scalar=h[:, j : j + 1],
                          in1=scores, op0=mybir.AluOpType.mult,
                          op1=mybir.AluOpType.add,
                      )

                  # ---- layernorm over free dim, * gamma/temp + beta/temp ----
                  stats = sm.tile([C, 6], F32)
                  nc.vector.bn_stats(out=stats, in_=scores)
                  mv = sm.tile([C, 2], F32)
                  nc.vector.bn_aggr(out=mv, in_=stats)
                  # rstd = exp(-0.5*ln(var+eps)).  Sqrt lives in a different
                  # ACT function table than Exp; using Ln+Exp keeps every
                  # activation in one table set (#6: ln/exp/copy/square) and
                  # avoids two ~1.3us table reloads per batch.  (Rsqrt on ACT
                  # is rejected by bass for accuracy reasons.)
                  lnv = sm.tile([C, 1], F32)
                  nc.scalar.activation(
                      out=lnv, in_=mv[:, 1:2],
                      func=mybir.ActivationFunctionType.Ln,
                      bias=eps_sb[:, 0:1], scale=1.0,
                  )
                  rstd = sm.tile([C, 1], F32)
                  nc.scalar.activation(
                      out=rstd, in_=lnv,
                      func=mybir.ActivationFunctionType.Exp, scale=-0.5,
                  )
                  snorm = wg.tile([C, C], F32)
                  # (scores - mu)*rstd*gamma' + beta'
                  nc.vector.scalar_tensor_tensor(
                      out=snorm, in0=scores, scalar=mv[:, 0:1], in1=gamma_sb[:],
                      op0=mybir.AluOpType.subtract, op1=mybir.AluOpType.mult,
                  )
                  nc.vector.scalar_tensor_tensor(
                      out=snorm, in0=snorm, scalar=rstd[:, 0:1], in1=beta_sb[:],
                      op0=mybir.AluOpType.mult, op1=mybir.AluOpType.add,
                  )

                  # ---- top-4 mask with lowest-index tie-break ----
                  m8 = sm.tile([C, 8], F32)
                  nc.vector.max(out=m8, in_=snorm)
                  t4 = t4p[:, 8 * b : 8 * b + 8]
                  nc.gpsimd.tensor_copy(t4[:, 0:4], m8[:, 0:4])
                  smarked = wg.tile([C, C], F32)
                  nc.vector.match_replace(
                      out=smarked, in_to_replace=t4, in_values=snorm,
                      imm_value=1e30,
                  )

                  # ---- masked softmax + identity ----
                  negm = sm.tile([C, 1], F32)
                  nc.gpsimd.tensor_scalar(
                      out=negm, in0=m8[:, 0:1], scalar1=-1.0, scalar2=None,
                      op0=mybir.AluOpType.mult,
                  )
                  e = wg.tile([C, C], F32)
                  nc.scalar.activation(
                      out=e, in_=snorm, func=mybir.ActivationFunctionType.Exp,
                      bias=negm[:, 0:1], scale=1.0,
                  )
                  den = sm.tile([C, 1], F32)
                  wun = wg.tile([C, C], F32)
                  nc.vector.scalar_tensor_tensor(
                      out=wun, in0=smarked, scalar=1e29, in1=e,
                      op0=mybir.AluOpType.is_ge, op1=mybir.AluOpType.mult,
                      accum_out=den[:, 0:1],
                  )
                  rden = sm.tile([C, 1], F32)
                  nc.vector.reciprocal(rden, den)
                  wfin = wg.tile([C, C], F32)
                  nc.vector.scalar_tensor_tensor(
                      out=wfin, in0=wun, scalar=rden[:, 0:1], in1=identity[:],
                      op0=mybir.AluOpType.mult, op1=mybir.AluOpType.add,
                  )

                  # ---- transpose W' (fp32) and cast stationary to fp16 ----
                  wT_ps = pstr.tile([C, C], F32)
                  nc.tensor.transpose(wT_ps[:], wfin[:], identity[:])
                  wT = wts.tile([C, C], F16, tag="wT")
                  nc.scalar.copy(wT, wT_ps[:])

                  # ---- channel-mix matmuls (fp16, 1 cyc/row) + evacuate ----
                  # two 512-col matmuls land in one 2-bank [C,1024] PSUM
                  # tile so each evacuation op moves 1024 columns (halves
                  # the per-op overhead on the evac engines)
                  o_t = oout.tile([C, L], F16, tag="o")
                  for jj in range(NCHUNK // 2):
                      pm = psmm.tile([C, 2 * MM_CHUNK], F32)
                      for h2 in range(2):
                          lo = (2 * jj + h2) * MM_CHUNK
                          nc.tensor.matmul(
                              pm[:, h2 * MM_CHUNK : (h2 + 1) * MM_CHUNK],
                              wT[:], x_t[:, lo : lo + MM_CHUNK],
                              start=True, stop=True,
                          )
                      dst = o_t[:, 2 * jj * MM_CHUNK : 2 * (jj + 1) * MM_CHUNK]
                      if jj < 3:
                          nc.scalar.copy(dst, pm[:])
                      else:
                          nc.vector.tensor_copy(dst, pm[:])

                  # ---- stream out (fp16, ACT HWDGE / gpsimd SWDGE rings) ----
                  (nc.scalar if b % 2 == 0 else nc.gpsimd).dma_start(
                      out=out[b], in_=o_t)

    nc.compile()
    return nc


def _get_nc():
    global _NC
    if _NC is None:
        nc = bacc.Bacc("TRN2", target_bir_lowering=False, debug=False)
        _NC = _emit(nc)
    return _NC


def kernel(x, w1, b1, w2, b2, gamma, beta, temperature):
    nc = _get_nc()
    x16 = np.ascontiguousarray(x, dtype=np.float16)
    rep = {
        "w1": np.ascontiguousarray(w1, dtype=np.float32),
        "b1": np.ascontiguousarray(b1, dtype=np.float32),
        "w2": np.ascontiguousarray(w2, dtype=np.float32),
        "b2": np.ascontiguousarray(b2, dtype=np.float32),
        "gamma": np.ascontiguousarray(gamma, dtype=np.float32),
        "beta": np.ascontiguousarray(beta, dtype=np.float32),
        "temperature": np.ascontiguousarray(temperature, dtype=np.float32),
    }
    in_maps = [
        {"x": x16[i * BS : (i + 1) * BS], **rep} for i in range(NCORES)
    ]
    res = run_bass_kernel_spmd(nc, in_maps, core_ids=list(range(NCORES)))
    return np.concatenate(
        [r["out"].astype(np.float32) for r in res.results], axis=0
    )
